# revision 11
# baseline (speedup 1.0000x reference)
"""Trainium2 Bass kernel for nn_Classifier_62311385530651.

Math: reference computes per-class ridge-projector distances
    dist[q,c] = ||q @ P_c - q||^2,  P_c = H_c^T (H_c H_c^T + lam I)^-1 H_c
    logits = -mean_res(dist);  out = row-minmax(logits)

Algebra: dist[q,c] = ||q||^2 - u S_c u^T  with u = q H_c^T and
S_c = A + lam A^2, A = (G + lam I)^-1, G = H_c H_c^T (n x n, n=20).
||q||^2 is class-independent -> cancels under the per-row min-max, and
min-max is invariant to positive scaling, so mean -> sum.
Factoring S_c = R_c^T R_c gives   score[q,c] = || q @ (R_c H_c)^T ||^2.

So the device kernel is a single (400 x 512) @ (512 x 1280) fp32r
matmul per core (batch-sharded 8 ways), elementwise square, segment-sum
over n=20, a tiny ones-block matmul to sum over res=25, and a per-row
min-max.  The (C*n x d) folded basis Htilde = R_c H_c is computed on
host from `high` (parameter preprocessing, O(C n^2 d) ~ 13 MFLOP).

Layout/perf notes:
- Host packs x_shard^T and Htilde^T into one (128, 4, 1680) array whose
  per-partition lines are DRAM-contiguous, so each DMA moves 13 KB
  descriptors instead of 6.7 KB rows (the HW DGE is descriptor-rate
  bound).
- The two halves go over the two HWDGE queues (SP + ACT) in parallel.
- PSUM tiles are (100, 3, 512) = 3 banks so one Square activation
  covers a whole m-chunk (fewer instructions -> fewer event-semaphore
  splits, which cost ~100 ns each on every engine).
- fp32r matmuls keep full fp32 storage but stream at bf16 rate.
"""

import os
import sys

for _p in ("/opt/trn_rl_repo", "/root/.axon_site/_ro/trn_rl_repo"):
    if os.path.isdir(_p) and _p not in sys.path:
        sys.path.append(_p)

import numpy as np

import concourse.bass as bass
import concourse.tile as tile
from concourse import bacc, mybir
from concourse.bass import ts
from concourse.bass_utils import run_bass_kernel_spmd

# Problem shapes (hardcoded per contest contract)
B_FULL, RES, D = 128, 25, 512
C, N = 64, 20
CN = C * N  # 1280
N_CORES = 8
B = B_FULL // N_CORES  # 16 batches per core
Q = B * RES  # 400 query rows per core
W = Q + CN  # 1680 combined free dim
KC = D // 128  # 4 contraction chunks
M = 100  # query-row chunk (4 batches * 25 res)
MC = Q // M  # 4
F_CHUNKS = [(0, 512), (512, 512), (1024, 256)]
FB = len(F_CHUNKS)  # psum banks per m-chunk

F32 = mybir.dt.float32
F32R = mybir.dt.float32r

MM_DT = F32 if os.environ.get("BASSK_MM_DTYPE") == "f32" else F32R


def build_nc():
    nc = bacc.Bacc()
    # (128, KC, W): line [p, k, :] = row k*128+p of [x_shard^T | Htilde^T]
    w_in = nc.dram_tensor("w_in", [128, KC, W], MM_DT, kind="ExternalInput")
    ones_w = nc.dram_tensor("ones_w", [M, C], F32, kind="ExternalInput")
    out = nc.dram_tensor("out", [B, C], F32, kind="ExternalOutput")

    with tile.TileContext(nc) as tc:
        with (
            tc.tile_pool(name="singles", bufs=1) as singles,
            tc.tile_pool(name="v2p", bufs=2) as v2p,
            tc.tile_pool(name="sp", bufs=MC) as sp,
            tc.tile_pool(name="small", bufs=1) as small,
            tc.tile_pool(name="psv", bufs=2, space="PSUM") as psv,
            tc.tile_pool(name="pst", bufs=1, space="PSUM") as pst,
            tc.tile_pool(name="psd", bufs=1, space="PSUM") as psd,
        ):
            wg = [
                singles.tile([128, 2, W], MM_DT, tag=f"wg{g}", name=f"wg{g}")
                for g in range(KC // 2)
            ]
            ones_sb = singles.tile([M, MC, B], F32)

            # ones first (tiny) so the PE touch clears immediately.
            nc.sync.dma_start(
                out=ones_sb[:, :, :],
                in_=ones_w[:, :].rearrange("p (m b) -> p m b", b=B),
            )
            # w halves: one per HWDGE queue, per-partition-contiguous lines.
            nc.sync.dma_start(out=wg[0][:, :, :], in_=w_in[:, 0:2, :])
            nc.scalar.dma_start(out=wg[1][:, :, :], in_=w_in[:, 2:4, :])

            # PE-touch ones_sb so the res-sum matmuls carry only their DVE
            # wait (matmul has a single wait slot before bacc splitting).
            dummy_ps = psd.tile([1, 1], F32)
            nc.tensor.matmul(
                dummy_ps[:, :],
                lhsT=ones_sb[:1, 0, :1],
                rhs=ones_sb[:1, 0, :1],
                start=True,
                stop=True,
            )

            s_tiles = []
            for m in range(MC):
                ps = psv.tile([M, FB, 512], F32)
                for fi, (f0, fs) in enumerate(F_CHUNKS):
                    for k in range(KC):
                        nc.tensor.matmul(
                            ps[:, fi, :fs],
                            lhsT=wg[k // 2][:, k % 2, ts(m, M)],
                            rhs=wg[k // 2][:, k % 2, Q + f0 : Q + f0 + fs],
                            start=(k == 0),
                            stop=(k == KC - 1),
                        )
                v2 = v2p.tile([M, FB * 512], F32)
                nc.scalar.activation(
                    out=v2[:, :],
                    in_=ps[:, :, :],
                    func=mybir.ActivationFunctionType.Square,
                )
                s_m = sp.tile([M, C], F32)
                nc.vector.tensor_reduce(
                    out=s_m[:, :],
                    in_=v2[:, :CN].rearrange("p (c n) -> p c n", n=N),
                    axis=mybir.AxisListType.X,
                    op=mybir.AluOpType.add,
                )
                s_tiles.append(s_m)

            t_ps = pst.tile([B, C], F32)
            for m in range(MC):
                nc.tensor.matmul(
                    t_ps[:, :],
                    lhsT=ones_sb[:, m, :],
                    rhs=s_tiles[m][:, :],
                    start=(m == 0),
                    stop=(m == MC - 1),
                )

            mn = small.tile([B, 1], F32)
            mx = small.tile([B, 1], F32)
            nc.vector.tensor_reduce(
                out=mn[:, :], in_=t_ps[:, :], axis=mybir.AxisListType.X,
                op=mybir.AluOpType.min,
            )
            nc.vector.tensor_reduce(
                out=mx[:, :], in_=t_ps[:, :], axis=mybir.AxisListType.X,
                op=mybir.AluOpType.max,
            )
            rng = small.tile([B, 1], F32)
            nc.vector.tensor_sub(rng[:, :], mx[:, :], mn[:, :])
            nc.vector.reciprocal(rng[:, :], rng[:, :])
            o_sb = small.tile([B, C], F32)
            nc.vector.tensor_scalar(
                out=o_sb[:, :],
                in0=t_ps[:, :],
                scalar1=mn[:, :],
                scalar2=rng[:, :],
                op0=mybir.AluOpType.subtract,
                op1=mybir.AluOpType.mult,
            )
            nc.sync.dma_start(out=out[:, :], in_=o_sb[:, :])
    nc.finalize()
    return nc


def _prep_params(high: np.ndarray):
    """Fold the ridge projector into Htilde (C*n, d): score = ||q Htilde_c^T||^2."""
    hi = np.asarray(high, dtype=np.float64)
    lam = N / D
    Ht = np.empty((C, N, D), dtype=np.float64)
    for c in range(C):
        H = hi[c]
        G = H @ H.T
        g, U = np.linalg.eigh(G)
        coef = np.sqrt(g + 2.0 * lam) / (g + lam)
        Ht[c] = (coef[:, None] * U.T) @ H
    return np.ascontiguousarray(
        Ht.reshape(CN, D).T, dtype=np.float32
    )  # (D, CN)


def _ones_block():
    w = np.zeros((M, MC, B), dtype=np.float32)
    for p in range(M):
        for m in range(MC):
            w[p, m, 4 * m + p // RES] = 1.0
    return np.ascontiguousarray(w.reshape(M, MC * B))


def run(x: np.ndarray, high: np.ndarray, **spmd_kwargs):
    x = np.asarray(x)
    assert x.shape == (B_FULL, RES, D)
    hT = _prep_params(high)
    ones_w = _ones_block()

    in_maps = []
    for i in range(N_CORES):
        xs = np.asarray(x[i * B : (i + 1) * B], dtype=np.float32).reshape(Q, D)
        w_core = np.concatenate([xs.T, hT], axis=1)  # (D, W)
        # partition-contiguous repack: w_host[p, k, :] = w_core[k*128+p, :]
        w_host = np.ascontiguousarray(
            w_core.reshape(KC, 128, W).transpose(1, 0, 2)
        )
        in_maps.append({"w_in": w_host, "ones_w": ones_w})

    nc = build_nc()
    res = run_bass_kernel_spmd(
        nc, in_maps, core_ids=list(range(N_CORES)), **spmd_kwargs
    )
    out = np.concatenate([r["out"] for r in res.results], axis=0)
    return out, res


def kernel(x: np.ndarray, high: np.ndarray) -> np.ndarray:
    return run(x, high)[0]


if __name__ == "__main__":
    rng = np.random.default_rng(0)
    x = rng.standard_normal((B_FULL, RES, D), dtype=np.float32)
    high = rng.standard_normal((C, N, D), dtype=np.float32)
    out = kernel(x=x, high=high)
    print(out.shape, out.dtype, out.min(), out.max())


# revision 14
# speedup vs baseline: 1.3133x; 1.3133x over previous
"""Trainium2 Bass kernel for nn_Classifier_62311385530651.

Math: reference computes per-class ridge-projector distances
    dist[q,c] = ||q @ P_c - q||^2,  P_c = H_c^T (H_c H_c^T + lam I)^-1 H_c
    logits = -mean_res(dist);  out = row-minmax(logits)

Algebra: dist[q,c] = ||q||^2 - u S_c u^T  with u = q H_c^T and
S_c = A + lam A^2, A = (G + lam I)^-1, G = H_c H_c^T (n x n, n=20).
||q||^2 is class-independent -> cancels under the per-row min-max, and
min-max is invariant to positive scaling, so mean -> sum.
Factoring S_c = R_c^T R_c gives   score[q,c] = || q @ (R_c H_c)^T ||^2.

So the device kernel is a single (400 x 512) @ (512 x 1280) fp32r
matmul per core (batch-sharded 8 ways), elementwise square, segment-sum
over n=20, a tiny ones-block matmul to sum over res=25, and a per-row
min-max.  The (C*n x d) folded basis Htilde = R_c H_c is computed on
host from `high` (parameter preprocessing, O(C n^2 d) ~ 13 MFLOP).

Layout/perf notes:
- Host packs x_shard^T and Htilde^T into one (128, 4, 1680) array whose
  per-partition lines are DRAM-contiguous, so each DMA moves 13 KB
  descriptors instead of 6.7 KB rows (the HW DGE is descriptor-rate
  bound).
- The two halves go over the two HWDGE queues (SP + ACT) in parallel.
- PSUM tiles are (100, 3, 512) = 3 banks so one Square activation
  covers a whole m-chunk (fewer instructions -> fewer event-semaphore
  splits, which cost ~100 ns each on every engine).
- fp32r matmuls keep full fp32 storage but stream at bf16 rate.
"""

import os
import sys

for _p in ("/opt/trn_rl_repo", "/root/.axon_site/_ro/trn_rl_repo"):
    if os.path.isdir(_p) and _p not in sys.path:
        sys.path.append(_p)

import numpy as np

import concourse.bass as bass
import concourse.tile as tile
from concourse import bacc, mybir
from concourse.bass import ts
from concourse.bass_utils import run_bass_kernel_spmd

# Problem shapes (hardcoded per contest contract)
B_FULL, RES, D = 128, 25, 512
C, N = 64, 20
CN = C * N  # 1280
N_CORES = 8
B = B_FULL // N_CORES  # 16 batches per core
Q = B * RES  # 400 query rows per core
W = Q + CN  # 1680 combined free dim
KC = D // 128  # 4 contraction chunks
M = 100  # query-row chunk (4 batches * 25 res)
MC = Q // M  # 4
F_CHUNKS = [(0, 512), (512, 512), (1024, 256)]
FB = len(F_CHUNKS)  # psum banks per m-chunk

F32 = mybir.dt.float32
F32R = mybir.dt.float32r
F16 = mybir.dt.float16

_MODE = os.environ.get("BASSK_MM_DTYPE", "f32r")
MM_DT = {"f32": F32, "f32r": F32R, "f16": F16}[_MODE]
MM_NP = np.float16 if _MODE == "f16" else np.float32


def build_nc():
    nc = bacc.Bacc()
    # (128, KC, W): line [p, k, :] = row k*128+p of [x_shard^T | Htilde^T]
    w_in = nc.dram_tensor("w_in", [128, KC, W], MM_DT, kind="ExternalInput")
    ones_w = nc.dram_tensor("ones_w", [M, C], F32, kind="ExternalInput")
    out = nc.dram_tensor("out", [B, C], F32, kind="ExternalOutput")

    with tile.TileContext(nc) as tc:
        with (
            tc.tile_pool(name="singles", bufs=1) as singles,
            tc.tile_pool(name="v2p", bufs=3) as v2p,
            tc.tile_pool(name="sp", bufs=MC) as sp,
            tc.tile_pool(name="small", bufs=1) as small,
            tc.tile_pool(name="psv", bufs=6, space="PSUM") as psv,
            tc.tile_pool(name="pst", bufs=1, space="PSUM") as pst,
            tc.tile_pool(name="psd", bufs=1, space="PSUM") as psd,
        ):
            wk = [
                singles.tile([128, W], MM_DT, tag=f"wk{k}", name=f"wk{k}")
                for k in range(KC)
            ]
            ones_sb = singles.tile([M, MC, B], F32)
            scratch = singles.tile([128, 512], F32)

            # PE warm-up fodder: no-input-dependency matmuls keep the PE HAM
            # clock at 2.4 GHz while the input DMAs stream in.
            nc.gpsimd.memset(scratch[:, :], 1.0)

            # ones first (tiny) so the PE touch clears immediately; then the
            # four k-chunk DMAs alternate over the two HWDGE queues (SP/ACT).
            nc.sync.dma_start(
                out=ones_sb[:, :, :],
                in_=ones_w[:, :].rearrange("p (m b) -> p m b", b=B),
            )
            for k in range(KC):
                eng = nc.sync if k % 2 == 0 else nc.scalar
                eng.dma_start(out=wk[k][:, :], in_=w_in[:, k, :])

            dummy_ps = psd.tile([128, 512], F32)
            for _ in range(6):
                nc.tensor.matmul(
                    dummy_ps[:, :],
                    lhsT=scratch[:, :128],
                    rhs=scratch[:, :],
                    start=True,
                    stop=True,
                )
            # PE-touch ones_sb so the res-sum matmuls carry only their DVE
            # wait (matmul has a single wait slot before bacc splitting).
            nc.tensor.matmul(
                dummy_ps[:1, :1],
                lhsT=ones_sb[:1, 0, :1],
                rhs=ones_sb[:1, 0, :1],
                start=True,
                stop=True,
            )

            s_tiles = []
            for m in range(MC):
                v2 = v2p.tile([M, CN], F32)
                for f0, fs in F_CHUNKS:
                    ps = psv.tile([M, 512], F32)
                    for k in range(KC):
                        nc.tensor.matmul(
                            ps[:, :fs],
                            lhsT=wk[k][:, ts(m, M)],
                            rhs=wk[k][:, Q + f0 : Q + f0 + fs],
                            start=(k == 0),
                            stop=(k == KC - 1),
                        )
                    nc.scalar.activation(
                        out=v2[:, f0 : f0 + fs],
                        in_=ps[:, :fs],
                        func=mybir.ActivationFunctionType.Square,
                    )
                s_m = sp.tile([M, C], F32)
                nc.vector.tensor_reduce(
                    out=s_m[:, :],
                    in_=v2.rearrange("p (c n) -> p c n", n=N),
                    axis=mybir.AxisListType.X,
                    op=mybir.AluOpType.add,
                )
                s_tiles.append(s_m)

            t_ps = pst.tile([B, C], F32)
            for m in range(MC):
                nc.tensor.matmul(
                    t_ps[:, :],
                    lhsT=ones_sb[:, m, :],
                    rhs=s_tiles[m][:, :],
                    start=(m == 0),
                    stop=(m == MC - 1),
                )

            mn = small.tile([B, 1], F32)
            mx = small.tile([B, 1], F32)
            nc.vector.tensor_reduce(
                out=mn[:, :], in_=t_ps[:, :], axis=mybir.AxisListType.X,
                op=mybir.AluOpType.min,
            )
            nc.vector.tensor_reduce(
                out=mx[:, :], in_=t_ps[:, :], axis=mybir.AxisListType.X,
                op=mybir.AluOpType.max,
            )
            rng = small.tile([B, 1], F32)
            nc.vector.tensor_sub(rng[:, :], mx[:, :], mn[:, :])
            nc.vector.reciprocal(rng[:, :], rng[:, :])
            o_sb = small.tile([B, C], F32)
            nc.vector.tensor_scalar(
                out=o_sb[:, :],
                in0=t_ps[:, :],
                scalar1=mn[:, :],
                scalar2=rng[:, :],
                op0=mybir.AluOpType.subtract,
                op1=mybir.AluOpType.mult,
            )
            nc.sync.dma_start(out=out[:, :], in_=o_sb[:, :])
    nc.finalize()
    return nc


def _prep_params(high: np.ndarray):
    """Fold the ridge projector into Htilde (C*n, d): score = ||q Htilde_c^T||^2."""
    hi = np.asarray(high, dtype=np.float64)
    lam = N / D
    Ht = np.empty((C, N, D), dtype=np.float64)
    for c in range(C):
        H = hi[c]
        G = H @ H.T
        g, U = np.linalg.eigh(G)
        coef = np.sqrt(g + 2.0 * lam) / (g + lam)
        Ht[c] = (coef[:, None] * U.T) @ H
    return np.ascontiguousarray(
        Ht.reshape(CN, D).T, dtype=np.float32
    )  # (D, CN)


def _ones_block():
    w = np.zeros((M, MC, B), dtype=np.float32)
    for p in range(M):
        for m in range(MC):
            w[p, m, 4 * m + p // RES] = 1.0
    return np.ascontiguousarray(w.reshape(M, MC * B))


def run(x: np.ndarray, high: np.ndarray, **spmd_kwargs):
    x = np.asarray(x)
    assert x.shape == (B_FULL, RES, D)
    hT = _prep_params(high)
    ones_w = _ones_block()

    in_maps = []
    for i in range(N_CORES):
        xs = np.asarray(x[i * B : (i + 1) * B], dtype=np.float32).reshape(Q, D)
        w_core = np.concatenate([xs.T, hT], axis=1)  # (D, W)
        # partition-contiguous repack: w_host[p, k, :] = w_core[k*128+p, :]
        w_host = np.ascontiguousarray(
            w_core.reshape(KC, 128, W).transpose(1, 0, 2), dtype=MM_NP
        )
        in_maps.append({"w_in": w_host, "ones_w": ones_w})

    nc = build_nc()
    res = run_bass_kernel_spmd(
        nc, in_maps, core_ids=list(range(N_CORES)), **spmd_kwargs
    )
    out = np.concatenate([r["out"] for r in res.results], axis=0)
    return out, res


def kernel(x: np.ndarray, high: np.ndarray) -> np.ndarray:
    return run(x, high)[0]


if __name__ == "__main__":
    rng = np.random.default_rng(0)
    x = rng.standard_normal((B_FULL, RES, D), dtype=np.float32)
    high = rng.standard_normal((C, N, D), dtype=np.float32)
    out = kernel(x=x, high=high)
    print(out.shape, out.dtype, out.min(), out.max())
